# revision 15
# baseline (speedup 1.0000x reference)
"""Trainium2 Bass kernel for nn_MultiHeadMLP.

Math:  out[b,h,s] = ELU(x[b,:] @ W1[h] + b1[h]) @ W2[h] + b2[h]
Shapes: x [131072, 64] f32, W1 [16, 64, 128], b1 [16, 128],
        W2 [16, 128, 64], b2 [16, 64]  ->  out [131072, 16, 64] f32.

Strategy (8 NeuronCores, batch data-parallel, 16384 rows/core):
  - Host pre-packs: xT augmented with a ones-row (K=65), W1 augmented with a
    (b1+1) row so MM1 emits z+1 directly; W2 flattened per head; output bias
    b2' = b2 - colsum(W2) folds the "-1" of ELU through MM2's linearity.
  - ELU identity used on-chip (exact):
        elu(z) + 1 = max(min(exp(z), 1), z + 1)
    so per head-pair group: PE MM1 -> psum1 = z+1; ACT: E = exp(psum1 - 1);
    DVE fused scalar_tensor_tensor: h' = (E min 1) max psum1;
    PE MM2 (col-tiled pair): psum2[s,b] = h' @ W2; ACT/DVE staging adds b2'
    and copies to SBUF; contiguous DMA to DRAM out laid out [H*S, B_shard]
    (host transposes back to [B, H, S] in numpy for free).
"""

import numpy as np

IN_SZ, HID_SZ, OUT_SZ, NUM_HEAD = 64, 128, 64, 16
BATCH = 131072
N_CORES = 8
B_SH = BATCH // N_CORES          # 16384 rows per core
KAUG = IN_SZ + 1                 # 65: augmented contraction dim
N_PAIR = NUM_HEAD // 2           # 8 head pairs

_PROG_CACHE = {}


def build_program(b_sh=B_SH, tile_b=512, act_stage_pairs=5, lookahead=2,
                  mm_f32r=True):
    """Emit the Bass/Tile program for one core processing b_sh batch rows.

    act_stage_pairs: head-pair groups (g mod 8) < act_stage_pairs stage their
    MM2 output on ScalarE (with bias), the rest on VectorE — load balancing
    the PSUM->SBUF exit between the two engines.
    lookahead: how many head-pair groups of MM1 are issued ahead of the
    ELU/MM2/staging chain (software pipelining; needs psum1 bufs=lookahead+1).
    mm_f32r: run matmuls in float32r (1 cycle/row on PE vs 4 for fp32).
    """
    import concourse.bacc as bacc
    import concourse.mybir as mybir
    from concourse.tile import TileContext

    f32 = mybir.dt.float32
    f32r = mybir.dt.float32r
    AF = mybir.ActivationFunctionType
    OP = mybir.AluOpType

    def mmcast(ap):
        return ap

    n_tiles = b_sh // tile_b
    assert b_sh % tile_b == 0

    nc = bacc.Bacc("TRN2", debug=False)
    xTa = nc.dram_tensor("xTa", [KAUG, b_sh], f32r if mm_f32r else f32, kind="ExternalInput").ap()
    w1a = nc.dram_tensor("w1a", [KAUG, NUM_HEAD * HID_SZ], f32r if mm_f32r else f32, kind="ExternalInput").ap()
    w2c = nc.dram_tensor("w2c", [HID_SZ, NUM_HEAD * 2 * OUT_SZ], f32r if mm_f32r else f32, kind="ExternalInput").ap()
    b2p = nc.dram_tensor("b2p", [2 * OUT_SZ, N_PAIR], f32, kind="ExternalInput").ap()
    out = nc.dram_tensor("out", [NUM_HEAD * OUT_SZ, b_sh], f32, kind="ExternalOutput").ap()

    # Register a -1.0 const AP (used as the Exp activation bias).
    neg1 = nc.alloc_sbuf_tensor("const-neg-one", [128, 1], f32)
    nc.gpsimd.memset(neg1.ap(), -1.0)
    nc.const_aps.aps[(f32, -1.0)] = neg1.ap()
    nc.all_engine_barrier()

    with TileContext(nc) as tc:
        with tc.tile_pool(name="const", bufs=1) as cpool, \
             tc.tile_pool(name="work", bufs=3) as wpool, \
             tc.tile_pool(name="outp", bufs=4) as opool, \
             tc.tile_pool(name="ps1", bufs=3, space="PSUM") as p1pool, \
             tc.tile_pool(name="ps2", bufs=2, space="PSUM") as p2pool:

            xTa_sb = cpool.tile([KAUG, b_sh], f32r if mm_f32r else f32)
            nc.sync.dma_start(out=xTa_sb, in_=xTa)
            w1_sb = cpool.tile([KAUG, NUM_HEAD * HID_SZ], f32r if mm_f32r else f32)
            nc.sync.dma_start(out=w1_sb, in_=w1a)
            w2_sb = cpool.tile([HID_SZ, NUM_HEAD * 2 * OUT_SZ], f32r if mm_f32r else f32)
            nc.sync.dma_start(out=w2_sb, in_=w2c)
            b2_sb = cpool.tile([2 * OUT_SZ, N_PAIR], f32)
            nc.sync.dma_start(out=b2_sb, in_=b2p)

            # Three-stage software pipeline over head-pair groups:
            #   stage A: MM1 pair -> psum1 (z+1)
            #   stage B: ACT exp + DVE fused combine -> hp
            #   stage C: MM2 accumulate pair -> psum2, staging (+b2'), DMA out
            def stage_a(t, g):
                xT_t = xTa_sb[:, t * tile_b:(t + 1) * tile_b]
                ps1 = p1pool.tile([HID_SZ, 2 * tile_b], f32, tag="p1")
                for j, h in enumerate((2 * g, 2 * g + 1)):
                    nc.tensor.matmul(
                        ps1[:, j * tile_b:(j + 1) * tile_b],
                        w1_sb[:, h * HID_SZ:(h + 1) * HID_SZ],
                        xT_t,
                        start=True, stop=True,
                    )
                return (t, g, ps1)

            def stage_b(st):
                t, g, ps1 = st
                E = wpool.tile([HID_SZ, 2 * tile_b], f32, tag="E")
                nc.scalar.activation(E, ps1, AF.Exp, bias=-1.0)
                hp = wpool.tile([HID_SZ, 2 * tile_b], f32r if mm_f32r else f32, tag="hp")
                nc.vector.scalar_tensor_tensor(hp, E, 1.0, ps1, OP.min, OP.max)
                return (t, g, hp)

            def stage_c(st):
                t, g, hp = st
                hA, hB = 2 * g, 2 * g + 1
                ps2 = p2pool.tile([2 * OUT_SZ, tile_b], f32, tag="p2")
                nc.tensor.matmul(
                    ps2,
                    w2_sb[:, hA * 2 * OUT_SZ:(hA + 1) * 2 * OUT_SZ],
                    hp[:, 0:tile_b],
                    start=True, stop=False,
                )
                nc.tensor.matmul(
                    ps2,
                    w2_sb[:, hB * 2 * OUT_SZ:(hB + 1) * 2 * OUT_SZ],
                    hp[:, tile_b:2 * tile_b],
                    start=False, stop=True,
                )
                ot = opool.tile([2 * OUT_SZ, tile_b], f32, tag="ot")
                if g % 8 < act_stage_pairs:
                    nc.scalar.activation(ot, ps2, AF.Identity, bias=b2_sb[:, g:g + 1])
                else:
                    nc.vector.tensor_scalar(ot, ps2, b2_sb[:, g:g + 1], None, OP.add)
                nc.sync.dma_start(
                    out=out[g * 2 * OUT_SZ:(g + 1) * 2 * OUT_SZ,
                            t * tile_b:(t + 1) * tile_b],
                    in_=ot,
                )

            # Per step: finish the oldest group first (its consumers are
            # ready), then the middle stage, then the new lookahead MM1s —
            # keeps each in-order engine queue serving unblocked work.
            from collections import deque
            qa, qb = deque(), deque()
            for t in range(n_tiles):
                for g in range(N_PAIR):
                    if len(qb) >= 1 and len(qa) >= lookahead:
                        stage_c(qb.popleft())
                    if len(qa) >= lookahead:
                        qb.append(stage_b(qa.popleft()))
                    qa.append(stage_a(t, g))
            while qa:
                if qb:
                    stage_c(qb.popleft())
                qb.append(stage_b(qa.popleft()))
            while qb:
                stage_c(qb.popleft())
    nc.compile()
    return nc


def _get_program():
    key = (B_SH, 512)
    if key not in _PROG_CACHE:
        _PROG_CACHE[key] = build_program(B_SH, 512)
    return _PROG_CACHE[key]


def pack_inputs(x, W1, b1, W2, b2, n_cores=N_CORES):
    """Host-side packing into per-core DRAM input maps."""
    x = np.ascontiguousarray(np.asarray(x, dtype=np.float32))
    W1 = np.asarray(W1, dtype=np.float32)
    b1 = np.asarray(b1, dtype=np.float32)
    W2 = np.asarray(W2, dtype=np.float32)
    b2 = np.asarray(b2, dtype=np.float32)

    w1a = np.empty((KAUG, NUM_HEAD * HID_SZ), np.float32)
    w1a[:IN_SZ] = W1.transpose(1, 0, 2).reshape(IN_SZ, NUM_HEAD * HID_SZ)
    w1a[IN_SZ] = (b1 + 1.0).reshape(-1)
    # Zero-padded per-head stationaries: head h occupies cols
    # h*128 + (h%2)*64 .. +64 so a pair accumulates into one [128,b] psum.
    w2c = np.zeros((HID_SZ, NUM_HEAD * 2 * OUT_SZ), np.float32)
    for h in range(NUM_HEAD):
        w2c[:, h * 2 * OUT_SZ + (h % 2) * OUT_SZ:
               h * 2 * OUT_SZ + (h % 2) * OUT_SZ + OUT_SZ] = W2[h]
    b2p_full = (b2 - W2.sum(axis=1)).reshape(-1)          # [H*S] = [1024]
    b2p = np.ascontiguousarray(b2p_full.reshape(N_PAIR, 2 * OUT_SZ).T)

    b_sh = x.shape[0] // n_cores
    in_maps = []
    for c in range(n_cores):
        xs = x[c * b_sh:(c + 1) * b_sh]
        xTa = np.empty((KAUG, b_sh), np.float32)
        xTa[:IN_SZ] = xs.T
        xTa[IN_SZ] = 1.0
        in_maps.append({"xTa": xTa, "w1a": w1a, "w2c": w2c, "b2p": b2p})
    return in_maps


def _install_ntff_hook():
    """Make trace=True work: register the axon NTFF profile hook that the
    container's antenv snapshot is missing (replicates trn_boot step 6)."""
    import sys, types
    try:
        from antenv.axon_hooks import get_axon_ntff_profile_hook  # noqa: F401
        return
    except ImportError:
        pass
    import antenv
    from trn_agent_boot.trn_boot import _ntff_profile_via_ctypes
    hook = _ntff_profile_via_ctypes("/opt/axon/libaxon_pjrt.so")
    mod = types.ModuleType("antenv.axon_hooks")
    mod.get_axon_ntff_profile_hook = lambda: hook
    mod.set_axon_ntff_profile_hook = lambda h: None
    sys.modules["antenv.axon_hooks"] = mod
    antenv.axon_hooks = mod


def run(x, W1, b1, W2, b2, trace=False):
    """Run on the 8 NeuronCores; returns (out [B,H,S], BassKernelResults)."""
    from concourse import bass_utils
    if trace:
        _install_ntff_hook()
    nc = _get_program()
    in_maps = pack_inputs(x, W1, b1, W2, b2)
    res = bass_utils.run_bass_kernel_spmd(
        nc, in_maps, core_ids=list(range(N_CORES)), trace=trace)
    outs = []
    for c in range(N_CORES):
        o = res.results[c]["out"]                          # [H*S, B_SH]
        outs.append(o.reshape(NUM_HEAD, OUT_SZ, B_SH).transpose(2, 0, 1))
    full = np.ascontiguousarray(np.concatenate(outs, axis=0))
    return full, res


def kernel(x, W1, b1, W2, b2):
    out, _ = run(x, W1, b1, W2, b2, trace=False)
    return out


# revision 17
# speedup vs baseline: 1.1252x; 1.1252x over previous
"""Trainium2 Bass kernel for nn_MultiHeadMLP.

Math:  out[b,h,s] = ELU(x[b,:] @ W1[h] + b1[h]) @ W2[h] + b2[h]
Shapes: x [131072, 64] f32, W1 [16, 64, 128], b1 [16, 128],
        W2 [16, 128, 64], b2 [16, 64]  ->  out [131072, 16, 64] f32.

Strategy (8 NeuronCores, batch data-parallel, 16384 rows/core):
  - Host pre-packs: xT augmented with a ones-row (K=65), W1 augmented with a
    (b1+1) row so MM1 emits z+1 directly; W2 flattened per head; output bias
    b2' = b2 - colsum(W2) folds the "-1" of ELU through MM2's linearity.
  - ELU identity used on-chip (exact):
        elu(z) + 1 = max(min(exp(z), 1), z + 1)
    so per head-pair group: PE MM1 -> psum1 = z+1; ACT: E = exp(psum1 - 1);
    DVE fused scalar_tensor_tensor: h' = (E min 1) max psum1;
    PE MM2 (col-tiled pair): psum2[s,b] = h' @ W2; ACT/DVE staging adds b2'
    and copies to SBUF; contiguous DMA to DRAM out laid out [H*S, B_shard]
    (host transposes back to [B, H, S] in numpy for free).
"""

import numpy as np

IN_SZ, HID_SZ, OUT_SZ, NUM_HEAD = 64, 128, 64, 16
BATCH = 131072
N_CORES = 8
B_SH = BATCH // N_CORES          # 16384 rows per core
KAUG = IN_SZ + 1                 # 65: augmented contraction dim
N_PAIR = NUM_HEAD // 2           # 8 head pairs

_PROG_CACHE = {}


def build_program(b_sh=B_SH, tile_b=512, act_stage_pairs=5, lookahead=2,
                  mm_f32r=True):
    """Emit the Bass/Tile program for one core processing b_sh batch rows.

    act_stage_pairs: head-pair groups (g mod 8) < act_stage_pairs stage their
    MM2 output on ScalarE (with bias), the rest on VectorE — load balancing
    the PSUM->SBUF exit between the two engines.
    lookahead: how many head-pair groups of MM1 are issued ahead of the
    ELU/MM2/staging chain (software pipelining; needs psum1 bufs=lookahead+1).
    mm_f32r: run matmuls in float32r (1 cycle/row on PE vs 4 for fp32).
    """
    import concourse.bacc as bacc
    import concourse.mybir as mybir
    from concourse.tile import TileContext

    f32 = mybir.dt.float32
    f32r = mybir.dt.float32r
    AF = mybir.ActivationFunctionType
    OP = mybir.AluOpType

    def mmcast(ap):
        return ap

    n_tiles = b_sh // tile_b
    assert b_sh % tile_b == 0

    nc = bacc.Bacc("TRN2", debug=False)
    xTa = nc.dram_tensor("xTa", [KAUG, b_sh], f32r if mm_f32r else f32, kind="ExternalInput").ap()
    w1a = nc.dram_tensor("w1a", [KAUG, NUM_HEAD * HID_SZ], f32r if mm_f32r else f32, kind="ExternalInput").ap()
    w2c = nc.dram_tensor("w2c", [HID_SZ, NUM_HEAD * 2 * OUT_SZ], f32r if mm_f32r else f32, kind="ExternalInput").ap()
    b2p = nc.dram_tensor("b2p", [2 * OUT_SZ, N_PAIR], f32, kind="ExternalInput").ap()
    out = nc.dram_tensor("out", [NUM_HEAD * OUT_SZ, b_sh], f32, kind="ExternalOutput").ap()

    # Register a -1.0 const AP (used as the Exp activation bias).
    neg1 = nc.alloc_sbuf_tensor("const-neg-one", [128, 1], f32)
    nc.gpsimd.memset(neg1.ap(), -1.0)
    nc.const_aps.aps[(f32, -1.0)] = neg1.ap()
    nc.all_engine_barrier()

    with TileContext(nc) as tc:
        with tc.tile_pool(name="const", bufs=1) as cpool, \
             tc.tile_pool(name="work", bufs=4) as wpool, \
             tc.tile_pool(name="outp", bufs=6) as opool, \
             tc.tile_pool(name="ps1", bufs=3, space="PSUM") as p1pool, \
             tc.tile_pool(name="ps2", bufs=2, space="PSUM") as p2pool:

            xTa_sb = cpool.tile([KAUG, b_sh], f32r if mm_f32r else f32)
            nc.sync.dma_start(out=xTa_sb, in_=xTa)
            w1_sb = cpool.tile([KAUG, NUM_HEAD * HID_SZ], f32r if mm_f32r else f32)
            nc.sync.dma_start(out=w1_sb, in_=w1a)
            w2_sb = cpool.tile([HID_SZ, NUM_HEAD * 2 * OUT_SZ], f32r if mm_f32r else f32)
            nc.sync.dma_start(out=w2_sb, in_=w2c)
            b2_sb = cpool.tile([2 * OUT_SZ, N_PAIR], f32)
            nc.sync.dma_start(out=b2_sb, in_=b2p)

            # Three-stage software pipeline over head-pair groups:
            #   stage A: MM1 pair -> psum1 (z+1)
            #   stage B: ACT exp + DVE fused combine -> hp
            #   stage C: MM2 accumulate pair -> psum2, staging (+b2'), DMA out
            def stage_a(t, g):
                xT_t = xTa_sb[:, t * tile_b:(t + 1) * tile_b]
                ps1 = p1pool.tile([HID_SZ, 2 * tile_b], f32, tag="p1")
                for j, h in enumerate((2 * g, 2 * g + 1)):
                    nc.tensor.matmul(
                        ps1[:, j * tile_b:(j + 1) * tile_b],
                        w1_sb[:, h * HID_SZ:(h + 1) * HID_SZ],
                        xT_t,
                        start=True, stop=True,
                    )
                return (t, g, ps1)

            def stage_b(st):
                t, g, ps1 = st
                E = wpool.tile([HID_SZ, 2 * tile_b], f32, tag="E")
                nc.scalar.activation(E, ps1, AF.Exp, bias=-1.0)
                hp = wpool.tile([HID_SZ, 2 * tile_b], f32r if mm_f32r else f32, tag="hp")
                nc.vector.scalar_tensor_tensor(hp, E, 1.0, ps1, OP.min, OP.max)
                return (t, g, hp)

            def stage_c(st):
                t, g, hp = st
                hA, hB = 2 * g, 2 * g + 1
                ps2 = p2pool.tile([2 * OUT_SZ, tile_b], f32, tag="p2")
                nc.tensor.matmul(
                    ps2,
                    w2_sb[:, hA * 2 * OUT_SZ:(hA + 1) * 2 * OUT_SZ],
                    hp[:, 0:tile_b],
                    start=True, stop=False,
                )
                nc.tensor.matmul(
                    ps2,
                    w2_sb[:, hB * 2 * OUT_SZ:(hB + 1) * 2 * OUT_SZ],
                    hp[:, tile_b:2 * tile_b],
                    start=False, stop=True,
                )
                ot = opool.tile([2 * OUT_SZ, tile_b], f32, tag="ot")
                if g % 8 < act_stage_pairs:
                    nc.scalar.activation(ot, ps2, AF.Identity, bias=b2_sb[:, g:g + 1])
                else:
                    nc.vector.tensor_scalar(ot, ps2, b2_sb[:, g:g + 1], None, OP.add)
                nc.sync.dma_start(
                    out=out[g * 2 * OUT_SZ:(g + 1) * 2 * OUT_SZ,
                            t * tile_b:(t + 1) * tile_b],
                    in_=ot,
                )

            from collections import deque
            qa, qb = deque(), deque()
            for t in range(n_tiles):
                for g in range(N_PAIR):
                    qa.append(stage_a(t, g))
                    if len(qa) > lookahead:
                        qb.append(stage_b(qa.popleft()))
                        if len(qb) > 1:
                            stage_c(qb.popleft())
            while qa:
                qb.append(stage_b(qa.popleft()))
                if len(qb) > 1:
                    stage_c(qb.popleft())
            while qb:
                stage_c(qb.popleft())
    nc.compile()
    return nc


def _get_program():
    key = (B_SH, 512)
    if key not in _PROG_CACHE:
        _PROG_CACHE[key] = build_program(B_SH, 512)
    return _PROG_CACHE[key]


def pack_inputs(x, W1, b1, W2, b2, n_cores=N_CORES):
    """Host-side packing into per-core DRAM input maps."""
    x = np.ascontiguousarray(np.asarray(x, dtype=np.float32))
    W1 = np.asarray(W1, dtype=np.float32)
    b1 = np.asarray(b1, dtype=np.float32)
    W2 = np.asarray(W2, dtype=np.float32)
    b2 = np.asarray(b2, dtype=np.float32)

    w1a = np.empty((KAUG, NUM_HEAD * HID_SZ), np.float32)
    w1a[:IN_SZ] = W1.transpose(1, 0, 2).reshape(IN_SZ, NUM_HEAD * HID_SZ)
    w1a[IN_SZ] = (b1 + 1.0).reshape(-1)
    # Zero-padded per-head stationaries: head h occupies cols
    # h*128 + (h%2)*64 .. +64 so a pair accumulates into one [128,b] psum.
    w2c = np.zeros((HID_SZ, NUM_HEAD * 2 * OUT_SZ), np.float32)
    for h in range(NUM_HEAD):
        w2c[:, h * 2 * OUT_SZ + (h % 2) * OUT_SZ:
               h * 2 * OUT_SZ + (h % 2) * OUT_SZ + OUT_SZ] = W2[h]
    b2p_full = (b2 - W2.sum(axis=1)).reshape(-1)          # [H*S] = [1024]
    b2p = np.ascontiguousarray(b2p_full.reshape(N_PAIR, 2 * OUT_SZ).T)

    b_sh = x.shape[0] // n_cores
    in_maps = []
    for c in range(n_cores):
        xs = x[c * b_sh:(c + 1) * b_sh]
        xTa = np.empty((KAUG, b_sh), np.float32)
        xTa[:IN_SZ] = xs.T
        xTa[IN_SZ] = 1.0
        in_maps.append({"xTa": xTa, "w1a": w1a, "w2c": w2c, "b2p": b2p})
    return in_maps


def _install_ntff_hook():
    """Make trace=True work: register the axon NTFF profile hook that the
    container's antenv snapshot is missing (replicates trn_boot step 6)."""
    import sys, types
    try:
        from antenv.axon_hooks import get_axon_ntff_profile_hook  # noqa: F401
        return
    except ImportError:
        pass
    import antenv
    from trn_agent_boot.trn_boot import _ntff_profile_via_ctypes
    hook = _ntff_profile_via_ctypes("/opt/axon/libaxon_pjrt.so")
    mod = types.ModuleType("antenv.axon_hooks")
    mod.get_axon_ntff_profile_hook = lambda: hook
    mod.set_axon_ntff_profile_hook = lambda h: None
    sys.modules["antenv.axon_hooks"] = mod
    antenv.axon_hooks = mod


def run(x, W1, b1, W2, b2, trace=False):
    """Run on the 8 NeuronCores; returns (out [B,H,S], BassKernelResults)."""
    from concourse import bass_utils
    if trace:
        _install_ntff_hook()
    nc = _get_program()
    in_maps = pack_inputs(x, W1, b1, W2, b2)
    res = bass_utils.run_bass_kernel_spmd(
        nc, in_maps, core_ids=list(range(N_CORES)), trace=trace)
    outs = []
    for c in range(N_CORES):
        o = res.results[c]["out"]                          # [H*S, B_SH]
        outs.append(o.reshape(NUM_HEAD, OUT_SZ, B_SH).transpose(2, 0, 1))
    full = np.ascontiguousarray(np.concatenate(outs, axis=0))
    return full, res


def kernel(x, W1, b1, W2, b2):
    out, _ = run(x, W1, b1, W2, b2, trace=False)
    return out


# revision 19
# speedup vs baseline: 1.2265x; 1.0900x over previous
"""Trainium2 Bass kernel for nn_MultiHeadMLP.

Math:  out[b,h,s] = ELU(x[b,:] @ W1[h] + b1[h]) @ W2[h] + b2[h]
Shapes: x [131072, 64] f32, W1 [16, 64, 128], b1 [16, 128],
        W2 [16, 128, 64], b2 [16, 64]  ->  out [131072, 16, 64] f32.

Strategy (8 NeuronCores, batch data-parallel, 16384 rows/core):
  - Host pre-packs: xT augmented with a ones-row (K=65), W1 augmented with a
    (b1+1) row so MM1 emits z+1 directly; W2 flattened per head; output bias
    b2' = b2 - colsum(W2) folds the "-1" of ELU through MM2's linearity.
  - ELU identity used on-chip (exact):
        elu(z) + 1 = max(min(exp(z), 1), z + 1)
    so per head-pair group: PE MM1 -> psum1 = z+1; ACT: E = exp(psum1 - 1);
    DVE fused scalar_tensor_tensor: h' = (E min 1) max psum1;
    PE MM2 (col-tiled pair): psum2[s,b] = h' @ W2; ACT/DVE staging adds b2'
    and copies to SBUF; contiguous DMA to DRAM out laid out [H*S, B_shard]
    (host transposes back to [B, H, S] in numpy for free).
"""

import numpy as np

IN_SZ, HID_SZ, OUT_SZ, NUM_HEAD = 64, 128, 64, 16
BATCH = 131072
N_CORES = 8
B_SH = BATCH // N_CORES          # 16384 rows per core
KAUG = IN_SZ + 1                 # 65: augmented contraction dim
N_PAIR = NUM_HEAD // 2           # 8 head pairs

_PROG_CACHE = {}


def build_program(b_sh=B_SH, tile_b=512, act_stage_pairs=5, lookahead=2,
                  mm_f32r=True):
    """Emit the Bass/Tile program for one core processing b_sh batch rows.

    act_stage_pairs: head-pair groups (g mod 8) < act_stage_pairs stage their
    MM2 output on ScalarE (with bias), the rest on VectorE — load balancing
    the PSUM->SBUF exit between the two engines.
    lookahead: how many head-pair groups of MM1 are issued ahead of the
    ELU/MM2/staging chain (software pipelining; needs psum1 bufs=lookahead+1).
    mm_f32r: run matmuls in float32r (1 cycle/row on PE vs 4 for fp32).
    """
    import concourse.bacc as bacc
    import concourse.mybir as mybir
    from concourse.tile import TileContext

    f32 = mybir.dt.float32
    f32r = mybir.dt.float32r
    AF = mybir.ActivationFunctionType
    OP = mybir.AluOpType

    def mmcast(ap):
        return ap

    n_tiles = b_sh // tile_b
    assert b_sh % tile_b == 0

    nc = bacc.Bacc("TRN2", debug=False)
    xTa = nc.dram_tensor("xTa", [KAUG, b_sh], f32r if mm_f32r else f32, kind="ExternalInput").ap()
    w1a = nc.dram_tensor("w1a", [KAUG, NUM_HEAD * HID_SZ], f32r if mm_f32r else f32, kind="ExternalInput").ap()
    w2c = nc.dram_tensor("w2c", [HID_SZ, NUM_HEAD * 2 * OUT_SZ], f32r if mm_f32r else f32, kind="ExternalInput").ap()
    b2p = nc.dram_tensor("b2p", [2 * OUT_SZ, N_PAIR], f32, kind="ExternalInput").ap()
    out = nc.dram_tensor("out", [NUM_HEAD * OUT_SZ, b_sh], f32, kind="ExternalOutput").ap()

    # Register a -1.0 const AP (used as the Exp activation bias).
    neg1 = nc.alloc_sbuf_tensor("const-neg-one", [128, 1], f32)
    nc.gpsimd.memset(neg1.ap(), -1.0)
    nc.const_aps.aps[(f32, -1.0)] = neg1.ap()
    nc.all_engine_barrier()

    with TileContext(nc) as tc:
        with tc.tile_pool(name="const", bufs=1) as cpool, \
             tc.tile_pool(name="work", bufs=4) as wpool, \
             tc.tile_pool(name="outp", bufs=6) as opool, \
             tc.tile_pool(name="ps1", bufs=3, space="PSUM") as p1pool, \
             tc.tile_pool(name="ps2", bufs=2, space="PSUM") as p2pool:

            xTa_sb = cpool.tile([KAUG, b_sh], f32r if mm_f32r else f32)
            for t in range(n_tiles):
                nc.sync.dma_start(
                    out=xTa_sb[:, t * tile_b:(t + 1) * tile_b],
                    in_=xTa[:, t * tile_b:(t + 1) * tile_b])
            w1_sb = cpool.tile([KAUG, NUM_HEAD * HID_SZ], f32r if mm_f32r else f32)
            nc.sync.dma_start(out=w1_sb, in_=w1a)
            w2_sb = cpool.tile([HID_SZ, NUM_HEAD * 2 * OUT_SZ], f32r if mm_f32r else f32)
            nc.sync.dma_start(out=w2_sb, in_=w2c)
            b2_sb = cpool.tile([2 * OUT_SZ, N_PAIR], f32)
            nc.sync.dma_start(out=b2_sb, in_=b2p)

            # Three-stage software pipeline over head-pair groups:
            #   stage A: MM1 pair -> psum1 (z+1)
            #   stage B: ACT exp + DVE fused combine -> hp
            #   stage C: MM2 accumulate pair -> psum2, staging (+b2'), DMA out
            def stage_a(t, g):
                xT_t = xTa_sb[:, t * tile_b:(t + 1) * tile_b]
                ps1 = p1pool.tile([HID_SZ, 2 * tile_b], f32, tag="p1")
                for j, h in enumerate((2 * g, 2 * g + 1)):
                    nc.tensor.matmul(
                        ps1[:, j * tile_b:(j + 1) * tile_b],
                        w1_sb[:, h * HID_SZ:(h + 1) * HID_SZ],
                        xT_t,
                        start=True, stop=True,
                    )
                return (t, g, ps1)

            def stage_b(st):
                t, g, ps1 = st
                E = wpool.tile([HID_SZ, 2 * tile_b], f32, tag="E")
                nc.scalar.activation(E, ps1, AF.Exp, bias=-1.0)
                hp = wpool.tile([HID_SZ, 2 * tile_b], f32r if mm_f32r else f32, tag="hp")
                nc.vector.scalar_tensor_tensor(hp, E, 1.0, ps1, OP.min, OP.max)
                return (t, g, hp)

            def stage_c(st):
                t, g, hp = st
                hA, hB = 2 * g, 2 * g + 1
                ps2 = p2pool.tile([2 * OUT_SZ, tile_b], f32, tag="p2")
                nc.tensor.matmul(
                    ps2,
                    w2_sb[:, hA * 2 * OUT_SZ:(hA + 1) * 2 * OUT_SZ],
                    hp[:, 0:tile_b],
                    start=True, stop=False,
                )
                nc.tensor.matmul(
                    ps2,
                    w2_sb[:, hB * 2 * OUT_SZ:(hB + 1) * 2 * OUT_SZ],
                    hp[:, tile_b:2 * tile_b],
                    start=False, stop=True,
                )
                ot = opool.tile([2 * OUT_SZ, tile_b], f32, tag="ot")
                # Spread the DVE-staged pairs through the tile rather than
                # clustering them (act_stage_pairs of 8 go to ScalarE).
                dve_set = {8: (), 7: (3,), 6: (2, 6), 5: (1, 4, 7),
                           4: (1, 3, 5, 7)}.get(act_stage_pairs, (1, 4, 7))
                if (g % 8) not in dve_set:
                    nc.scalar.activation(ot, ps2, AF.Identity, bias=b2_sb[:, g:g + 1])
                else:
                    nc.vector.tensor_scalar(ot, ps2, b2_sb[:, g:g + 1], None, OP.add)
                nc.sync.dma_start(
                    out=out[g * 2 * OUT_SZ:(g + 1) * 2 * OUT_SZ,
                            t * tile_b:(t + 1) * tile_b],
                    in_=ot,
                )

            from collections import deque
            qa, qb = deque(), deque()
            for t in range(n_tiles):
                for g in range(N_PAIR):
                    qa.append(stage_a(t, g))
                    if len(qa) > lookahead:
                        qb.append(stage_b(qa.popleft()))
                        if len(qb) > 1:
                            stage_c(qb.popleft())
            while qa:
                qb.append(stage_b(qa.popleft()))
                if len(qb) > 1:
                    stage_c(qb.popleft())
            while qb:
                stage_c(qb.popleft())
    nc.compile()
    return nc


def _get_program():
    key = (B_SH, 512)
    if key not in _PROG_CACHE:
        _PROG_CACHE[key] = build_program(B_SH, 512)
    return _PROG_CACHE[key]


def pack_inputs(x, W1, b1, W2, b2, n_cores=N_CORES):
    """Host-side packing into per-core DRAM input maps."""
    x = np.ascontiguousarray(np.asarray(x, dtype=np.float32))
    W1 = np.asarray(W1, dtype=np.float32)
    b1 = np.asarray(b1, dtype=np.float32)
    W2 = np.asarray(W2, dtype=np.float32)
    b2 = np.asarray(b2, dtype=np.float32)

    w1a = np.empty((KAUG, NUM_HEAD * HID_SZ), np.float32)
    w1a[:IN_SZ] = W1.transpose(1, 0, 2).reshape(IN_SZ, NUM_HEAD * HID_SZ)
    w1a[IN_SZ] = (b1 + 1.0).reshape(-1)
    # Zero-padded per-head stationaries: head h occupies cols
    # h*128 + (h%2)*64 .. +64 so a pair accumulates into one [128,b] psum.
    w2c = np.zeros((HID_SZ, NUM_HEAD * 2 * OUT_SZ), np.float32)
    for h in range(NUM_HEAD):
        w2c[:, h * 2 * OUT_SZ + (h % 2) * OUT_SZ:
               h * 2 * OUT_SZ + (h % 2) * OUT_SZ + OUT_SZ] = W2[h]
    b2p_full = (b2 - W2.sum(axis=1)).reshape(-1)          # [H*S] = [1024]
    b2p = np.ascontiguousarray(b2p_full.reshape(N_PAIR, 2 * OUT_SZ).T)

    b_sh = x.shape[0] // n_cores
    in_maps = []
    for c in range(n_cores):
        xs = x[c * b_sh:(c + 1) * b_sh]
        xTa = np.empty((KAUG, b_sh), np.float32)
        xTa[:IN_SZ] = xs.T
        xTa[IN_SZ] = 1.0
        in_maps.append({"xTa": xTa, "w1a": w1a, "w2c": w2c, "b2p": b2p})
    return in_maps


def _install_ntff_hook():
    """Make trace=True work: register the axon NTFF profile hook that the
    container's antenv snapshot is missing (replicates trn_boot step 6)."""
    import sys, types
    try:
        from antenv.axon_hooks import get_axon_ntff_profile_hook  # noqa: F401
        return
    except ImportError:
        pass
    import antenv
    from trn_agent_boot.trn_boot import _ntff_profile_via_ctypes
    hook = _ntff_profile_via_ctypes("/opt/axon/libaxon_pjrt.so")
    mod = types.ModuleType("antenv.axon_hooks")
    mod.get_axon_ntff_profile_hook = lambda: hook
    mod.set_axon_ntff_profile_hook = lambda h: None
    sys.modules["antenv.axon_hooks"] = mod
    antenv.axon_hooks = mod


def run(x, W1, b1, W2, b2, trace=False):
    """Run on the 8 NeuronCores; returns (out [B,H,S], BassKernelResults)."""
    from concourse import bass_utils
    if trace:
        _install_ntff_hook()
    nc = _get_program()
    in_maps = pack_inputs(x, W1, b1, W2, b2)
    res = bass_utils.run_bass_kernel_spmd(
        nc, in_maps, core_ids=list(range(N_CORES)), trace=trace)
    outs = []
    for c in range(N_CORES):
        o = res.results[c]["out"]                          # [H*S, B_SH]
        outs.append(o.reshape(NUM_HEAD, OUT_SZ, B_SH).transpose(2, 0, 1))
    full = np.ascontiguousarray(np.concatenate(outs, axis=0))
    return full, res


def kernel(x, W1, b1, W2, b2):
    out, _ = run(x, W1, b1, W2, b2, trace=False)
    return out
